# revision 11
# baseline (speedup 1.0000x reference)
"""Trainium2 Bass kernel for the ACTP 2-layer-LSTM + MLP rollout model.

Strategy: pure data parallel across 8 NeuronCores (batch 4096 -> 512/core),
weights replicated.  All on-chip tensors are feature-major [feat, batch] so
the time recurrence needs no transposes: matmuls are out[M,N] = W_T[K,M].T @
x[K,N] with the batch as the moving free dim (N=512), gate activations are
batched reads across PSUM banks, and every concat in the model becomes extra
K-chunk matmuls accumulating into the same PSUM bank.  Biases ride in padded
"ones-row" K-chunks (gates, fc1) or the activation bias operand (fc2).  The
tiled(act,state) input of LSTM2 collapses algebraically into a single padded
K=100 chunk.  ALL matmul K-chunks are padded to K=100: matmuls with K <= ~64
in the stream permanently block the PE HAM clock-gate from reaching 2.4 GHz
(measured: mixing K=48 or K=13 pins every matmul at cold ~512ns cadence).

Only tactiles[0:10] is ever read (the model feeds back its own output after
the context window), so device I/O is tiny.  Host does all transposes.
"""

import os
import sys
import functools

sys.path.insert(0, "/opt/trn_rl_repo")

import numpy as np
import ml_dtypes

import concourse.bass as bass
from concourse import bacc
import concourse.tile as tile
from concourse import mybir
from concourse.bass_utils import run_bass_kernel_spmd

# model dims
T = 120
B = 4096
F = 48   # tactile feature size
A = 6    # action dim
H = 200  # LSTM hidden
CTX = 10
NSTEP = T - 1            # 119 scan steps
NOUT = NSTEP - (CTX - 1)  # 110 outputs
NCORES = 8
BL = B // NCORES         # 512 per-core batch
HC = 100                 # H partition chunk (also the universal matmul K)
G4 = 4 * H               # 800 gate rows

COMPUTE_BF16 = True

LAST_RESULT = None  # BassKernelResults of the most recent run (for test.py)

Tanh = mybir.ActivationFunctionType.Tanh
Sigmoid = mybir.ActivationFunctionType.Sigmoid


def _dt():
    return mybir.dt.bfloat16 if COMPUTE_BF16 else mybir.dt.float32


def _npdt():
    return ml_dtypes.bfloat16 if COMPUTE_BF16 else np.float32


def _build_nc():
    nc = bacc.Bacc()
    dt = _dt()
    f32 = mybir.dt.float32

    # ---- DRAM parameters (per-core shards / replicated weights) ----
    # tact: [100, CTX, BL]: rows 0..47 tactile features, row 48 ones, rest 0
    tact = nc.declare_dram_parameter("tact", [HC, CTX, BL], dt, isOutput=False)
    acts = nc.declare_dram_parameter("acts", [A, NSTEP, BL], dt, isOutput=False)
    # statpad: [94, BL]: rows 0..5 state, row 6 ones, rows 7.. zeros
    #   (DMA'd into ast rows 6..99: state at 6..11, ones at 12, zeros 13..99)
    statpad = nc.declare_dram_parameter("statpad", [HC - A, BL], dt, isOutput=False)

    w1x = nc.declare_dram_parameter("w1x", [HC, G4], dt, isOutput=False)
    w1ha = nc.declare_dram_parameter("w1ha", [HC, G4], dt, isOutput=False)
    w1hb = nc.declare_dram_parameter("w1hb", [HC, G4], dt, isOutput=False)
    w2as = nc.declare_dram_parameter("w2as", [HC, G4], dt, isOutput=False)
    w2h1a = nc.declare_dram_parameter("w2h1a", [HC, G4], dt, isOutput=False)
    w2h1b = nc.declare_dram_parameter("w2h1b", [HC, G4], dt, isOutput=False)
    w2h2a = nc.declare_dram_parameter("w2h2a", [HC, G4], dt, isOutput=False)
    w2h2b = nc.declare_dram_parameter("w2h2b", [HC, G4], dt, isOutput=False)
    w3x = nc.declare_dram_parameter("w3x", [HC, H], dt, isOutput=False)
    w3ha = nc.declare_dram_parameter("w3ha", [HC, H], dt, isOutput=False)
    w3hb = nc.declare_dram_parameter("w3hb", [HC, H], dt, isOutput=False)
    w4a = nc.declare_dram_parameter("w4a", [HC, F], dt, isOutput=False)
    w4b = nc.declare_dram_parameter("w4b", [HC, F], dt, isOutput=False)
    b4 = nc.declare_dram_parameter("b4", [F, 1], f32, isOutput=False)

    out = nc.declare_dram_parameter("out", [NOUT, F, BL], f32, isOutput=True)

    from contextlib import ExitStack

    with tile.TileContext(nc) as tc, ExitStack() as ctx:
        # ---- pools ----
        wpool = ctx.enter_context(tc.tile_pool(name="wpool", bufs=1))
        stp = ctx.enter_context(tc.tile_pool(name="stp", bufs=1))
        sp = ctx.enter_context(tc.tile_pool(name="sp", bufs=2))
        op = ctx.enter_context(tc.tile_pool(name="op", bufs=4))
        pp = ctx.enter_context(tc.tile_pool(name="pp", bufs=4, space="PSUM"))

        # ---- weights to SBUF (once) ----
        W1X = wpool.tile([HC, G4], dt, name="W1X")
        W1HA = wpool.tile([HC, G4], dt, name="W1HA")
        W1HB = wpool.tile([HC, G4], dt, name="W1HB")
        W2AS = wpool.tile([HC, G4], dt, name="W2AS")
        W2H1A = wpool.tile([HC, G4], dt, name="W2H1A")
        W2H1B = wpool.tile([HC, G4], dt, name="W2H1B")
        W2H2A = wpool.tile([HC, G4], dt, name="W2H2A")
        W2H2B = wpool.tile([HC, G4], dt, name="W2H2B")
        W3X = wpool.tile([HC, H], dt, name="W3X")
        W3HA = wpool.tile([HC, H], dt, name="W3HA")
        W3HB = wpool.tile([HC, H], dt, name="W3HB")
        W4A = wpool.tile([HC, F], dt, name="W4A")
        W4B = wpool.tile([HC, F], dt, name="W4B")
        B4 = wpool.tile([F, 1], f32, name="B4")
        for sb, dr in [
            (W1X, w1x), (W1HA, w1ha), (W1HB, w1hb), (W2AS, w2as),
            (W2H1A, w2h1a), (W2H1B, w2h1b), (W2H2A, w2h2a), (W2H2B, w2h2b),
            (W3X, w3x), (W3HA, w3ha), (W3HB, w3hb), (W4A, w4a), (W4B, w4b),
            (B4, b4),
        ]:
            nc.sync.dma_start(out=sb, in_=dr[:, :])

        # ---- persistent state ----
        h1 = stp.tile([HC, 2, BL], dt, name="h1")
        h2 = stp.tile([HC, 2, BL], dt, name="h2")
        o3 = stp.tile([HC, 2, BL], dt, name="o3")
        c1 = stp.tile([HC, 2, BL], f32, name="c1")
        c2 = stp.tile([HC, 2, BL], f32, name="c2")
        # x1: rows 0..47 = inp feedback, row 48 = ones (bias ride), 49.. = 0
        x1 = stp.tile([HC, BL], dt, name="x1")
        TACT = stp.tile([HC, CTX, BL], dt, name="TACT")
        ACTS = stp.tile([A, NSTEP, BL], dt, name="ACTS")
        ast0 = stp.tile([HC, BL], dt, name="ast0")
        ast1 = stp.tile([HC, BL], dt, name="ast1")
        nc.sync.dma_start(out=TACT, in_=tact[:, :, :])
        nc.sync.dma_start(out=ACTS, in_=acts[:, :, :])
        # static rows of ast: state(6..11), ones(12), zeros(13..99)
        nc.sync.dma_start(out=ast0[A:HC, :], in_=statpad[:, :])
        nc.sync.dma_start(out=ast1[A:HC, :], in_=statpad[:, :])
        # x1 static rows: ones row at 48 + zero tail (statpad rows 6..57)
        nc.sync.dma_start(out=x1[F:HC, :], in_=statpad[A:A + (HC - F), :])

        nc.vector.memset(h1, 0.0)
        nc.vector.memset(h2, 0.0)
        nc.vector.memset(c1, 0.0)
        nc.vector.memset(c2, 0.0)

        h1a = h1[:, 0, :]
        h1b = h1[:, 1, :]
        h2a = h2[:, 0, :]
        h2b = h2[:, 1, :]

        # permuted gate row layout: [i(0:200) f(200:400) o(400:600) g(600:800)]
        COLBASE = {"i": 0, "f": 200, "o": 400, "g": 600}

        def lstm_gates(kchunks, tag):
            """Emit per-gate 2-bank PSUM tiles in order [g, i, f, o] (g first:
            tanh(g) heads the elementwise chain).  kchunks: list of
            (weight_tile, rhs_ap); accumulation runs in list order."""
            P = {}
            for gate in ("g", "i", "f", "o"):
                gp = pp.tile([HC, 2, BL], f32, name=f"P{gate}_{tag}", tag="g")
                for m in range(2):
                    col = COLBASE[gate] + m * HC
                    last = len(kchunks) - 1
                    for j, (W, rhs) in enumerate(kchunks):
                        nc.tensor.matmul(gp[:, m, :], W[:, col:col + HC], rhs,
                                         start=(j == 0), stop=(j == last))
                P[gate] = gp
            return P

        def lstm1_hparts(P, gates, tag):
            """Allocate LSTM1 gate tiles + emit the h-recurrence part of the
            accumulation (software pipelining: emitted before the previous
            step's fc block so the PE fills the fc dependency stall)."""
            for gate in gates:
                gp = pp.tile([HC, 2, BL], f32, name=f"P{gate}_{tag}", tag="g")
                for m in range(2):
                    col = COLBASE[gate] + m * HC
                    nc.tensor.matmul(gp[:, m, :], W1HA[:, col:col + HC], h1a,
                                     start=True, stop=False)
                    nc.tensor.matmul(gp[:, m, :], W1HB[:, col:col + HC], h1b,
                                     start=False, stop=False)
                P[gate] = gp

        def lstm1_xparts(P, x1_rhs):
            for gate in ("g", "i", "f", "o"):
                for m in range(2):
                    col = COLBASE[gate] + m * HC
                    nc.tensor.matmul(P[gate][:, m, :], W1X[:, col:col + HC],
                                     x1_rhs, start=False, stop=True)

        def lstm_cell(P, c, h, tag):
            gt = sp.tile([HC, 2, BL], dt, name=f"gt{tag}", tag="gt")
            sgi = sp.tile([HC, 2, BL], dt, name=f"sgi{tag}", tag="sgi")
            sgf = sp.tile([HC, 2, BL], dt, name=f"sgf{tag}", tag="sgf")
            sgo = sp.tile([HC, 2, BL], dt, name=f"sgo{tag}", tag="sgo")
            nc.scalar.activation(gt, P["g"], Tanh)
            nc.scalar.activation(sgi, P["i"], Sigmoid)
            nc.scalar.activation(sgf, P["f"], Sigmoid)
            nc.scalar.activation(sgo, P["o"], Sigmoid)
            ig = sp.tile([HC, 2, BL], dt, name=f"ig{tag}", tag="ig")
            fm = sp.tile([HC, 2, BL], f32, name=f"fm{tag}", tag="fm")
            nc.vector.tensor_mul(ig, sgi, gt)
            nc.vector.tensor_mul(fm, sgf, c)
            nc.vector.tensor_add(c, fm, ig)
            tch = sp.tile([HC, 2, BL], dt, name=f"tch{tag}", tag="tch")
            nc.scalar.activation(tch, c, Tanh)
            nc.vector.tensor_mul(h, sgo, tch)

        pend1 = None
        for t in range(NSTEP):
            x1_rhs = TACT[:, t, :] if t < CTX else x1
            ast = ast0 if t % 2 == 0 else ast1
            # refresh the act rows (0..5) for this step; same partition base
            nc.vector.tensor_copy(ast[0:A, :], ACTS[:, t, :])

            # ---- LSTM1 gates ----
            if pend1 is None:
                P1 = {}
                lstm1_hparts(P1, ("g", "i", "f", "o"), f"1_{t}")
            else:
                P1 = pend1
            lstm1_xparts(P1, x1_rhs)
            lstm_cell(P1, c1, h1, f"1_{t}")

            # ---- LSTM2 gates ----
            P2 = lstm_gates([(W2H2A, h2a), (W2H2B, h2b), (W2AS, ast),
                             (W2H1A, h1a), (W2H1B, h1b)], f"2_{t}")
            lstm_cell(P2, c2, h2, f"2_{t}")

            # software pipeline: next step's LSTM1 h-part matmuls (g,i now;
            # f,o after the fc tiles grab their PSUM slots)
            pend1 = {} if t + 1 < NSTEP else None
            if pend1 is not None:
                lstm1_hparts(pend1, ("g", "i"), f"1_{t + 1}")

            # ---- MLP head (only needed from t = CTX-1 on) ----
            if t >= CTX - 1:
                fcp = pp.tile([HC, 2, BL], f32, name=f"fcp_{t}", tag="g")
                for m in range(2):
                    ps = fcp[:, m, :]
                    ws = slice(m * HC, (m + 1) * HC)
                    nc.tensor.matmul(ps, W3X[:, ws], x1_rhs, start=True, stop=False)
                    nc.tensor.matmul(ps, W3HA[:, ws], h2a, start=False, stop=False)
                    nc.tensor.matmul(ps, W3HB[:, ws], h2b, start=False, stop=True)
                nc.scalar.activation(o3, fcp, Tanh)
                f2p = pp.tile([F, BL], f32, name=f"f2p_{t}", tag="g")
                p4 = f2p[:, :]
                nc.tensor.matmul(p4, W4A, o3[:, 0, :], start=True, stop=False)
                nc.tensor.matmul(p4, W4B, o3[:, 1, :], start=False, stop=True)
                stg = op.tile([F, BL], f32, name=f"stg_{t}", tag="stg")
                if t < NSTEP - 1:
                    # feedback: next step's input (fc2 bias via ACT bias operand)
                    nc.scalar.activation(x1[0:F, :], p4, Tanh, bias=B4)
                    # fp32 output staged from the bf16 feedback via DVE
                    nc.vector.tensor_copy(stg, x1[0:F, :])
                else:
                    nc.scalar.activation(stg, p4, Tanh, bias=B4)
                nc.gpsimd.dma_start(out=out[t - (CTX - 1)], in_=stg)
            if pend1 is not None:
                lstm1_hparts(pend1, ("f", "o"), f"1_{t + 1}")

    nc.finalize()
    return nc


@functools.lru_cache(maxsize=1)
def _get_nc():
    return _build_nc()


def _prep_weights(W_ih1, W_hh1, b_ih1, b_hh1, W_ih2, W_hh2, b_ih2, b_hh2,
                  fc1_w, fc1_b, fc2_w, fc2_b):
    npdt = _npdt()
    # gate rows reordered [i, f, o, g] so chunk order is [i0 i1 f0 f1 o0 o1 g0 g1]
    perm = np.concatenate([np.arange(0, 200), np.arange(200, 400),
                           np.arange(600, 800), np.arange(400, 600)])
    W1p = np.asarray(W_ih1)[perm]          # [800, 48]
    W1hp = np.asarray(W_hh1)[perm]         # [800, 200]
    b1p = (np.asarray(b_ih1) + np.asarray(b_hh1))[perm]
    W2p = np.asarray(W_ih2)[perm]          # [800, 248]
    W2hp = np.asarray(W_hh2)[perm]         # [800, 200]
    b2p = (np.asarray(b_ih2) + np.asarray(b_hh2))[perm]
    Wt = W2p[:, 200:248]
    W2eff = Wt[:, 0:12] + Wt[:, 12:24] + Wt[:, 24:36] + Wt[:, 36:48]  # [800, 12]
    fc1_w = np.asarray(fc1_w); fc1_b = np.asarray(fc1_b)
    fc2_w = np.asarray(fc2_w); fc2_b = np.asarray(fc2_b)

    def c(x):
        return np.ascontiguousarray(x).astype(npdt)

    def padK(x):
        k, m = x.shape
        z = np.zeros((HC, m), x.dtype)
        z[:k] = x
        return z

    # x-chunk weights: rows 0..47 = input features, row 48 = bias, rest 0
    w1x = padK(np.concatenate([W1p.T, b1p[None, :]], 0))          # [100, 800]
    w3x = padK(np.concatenate([fc1_w[:, 200:248].T, fc1_b[None, :]], 0))
    # act/state chunk: rows 0..5 act, 6..11 state, row 12 bias, rest 0
    w2as = padK(np.concatenate([W2eff.T, b2p[None, :]], 0))       # [100, 800]

    return {
        "w1x": c(w1x),
        "w1ha": c(W1hp[:, 0:100].T),
        "w1hb": c(W1hp[:, 100:200].T),
        "w2as": c(w2as),
        "w2h1a": c(W2p[:, 0:100].T),
        "w2h1b": c(W2p[:, 100:200].T),
        "w2h2a": c(W2hp[:, 0:100].T),
        "w2h2b": c(W2hp[:, 100:200].T),
        "w3x": c(w3x),
        "w3ha": c(fc1_w[:, 0:100].T),
        "w3hb": c(fc1_w[:, 100:200].T),
        "w4a": c(fc2_w[:, 0:100].T),
        "w4b": c(fc2_w[:, 100:200].T),
        "b4": np.ascontiguousarray(fc2_b[:, None]).astype(np.float32),
    }


def kernel(tactiles, actions, W_ih1, W_hh1, b_ih1, b_hh1,
           W_ih2, W_hh2, b_ih2, b_hh2, fc1_w, fc1_b, fc2_w, fc2_b):
    global LAST_RESULT
    npdt = _npdt()
    tactiles = np.asarray(tactiles)
    actions = np.asarray(actions)

    wmap = _prep_weights(W_ih1, W_hh1, b_ih1, b_hh1, W_ih2, W_hh2, b_ih2, b_hh2,
                         fc1_w, fc1_b, fc2_w, fc2_b)

    in_maps = []
    for i in range(NCORES):
        s = slice(i * BL, (i + 1) * BL)
        # tact: [100, CTX, BL] with row 48 = ones (bias ride), rest 0
        tt = np.zeros((HC, CTX, BL), np.float32)
        tt[0:F] = tactiles[0:CTX, s, :].transpose(2, 0, 1)
        tt[F] = 1.0
        acts_T = np.ascontiguousarray(
            actions[1:T, s, :].transpose(2, 0, 1)).astype(npdt)      # [6, 119, BL]
        # statpad rows (land at ast rows 6..99): state(6), ones(1), zeros
        sp_ = np.zeros((HC - A, BL), np.float32)
        sp_[0:A] = actions[0, s, :].T
        sp_[A] = 1.0
        m = {"tact": tt.astype(npdt), "acts": acts_T,
             "statpad": sp_.astype(npdt)}
        m.update(wmap)
        in_maps.append(m)

    nc = _get_nc()
    res = run_bass_kernel_spmd(nc, in_maps, core_ids=list(range(NCORES)))
    LAST_RESULT = res

    outs = [np.asarray(r["out"], dtype=np.float32) for r in res.results]
    # [NOUT, F, BL] per core -> [NOUT, B, F]
    full = np.concatenate([o.transpose(0, 2, 1) for o in outs], axis=1)
    return np.ascontiguousarray(full)


# revision 12
# speedup vs baseline: 1.0856x; 1.0856x over previous
"""Trainium2 Bass kernel for the ACTP 2-layer-LSTM + MLP rollout model.

Strategy: pure data parallel across 8 NeuronCores (batch 4096 -> 512/core),
weights replicated.  All on-chip tensors are feature-major [feat, batch] so
the time recurrence needs no transposes: matmuls are out[M,N] = W_T[K,M].T @
x[K,N] with the batch as the moving free dim (N=512).  Every concat in the
model becomes extra K-chunk matmuls accumulating into the same PSUM bank,
biases ride in "ones-row" K-chunks or the activation bias operand, and the
tiled(act,state) input of LSTM2 collapses algebraically into rows of the
h2-tail K-chunk.

Layout rules discovered on hardware:
 - matmuls with K <= ~64 anywhere in the stream permanently block the PE HAM
   clock-gate from reaching 2.4 GHz -> every K-chunk is padded to >= 96 rows
   (zero weight rows; rhs pad rows zeroed so 0*0 can't make NaN).
 - compute-engine writes at a partition offset must be 32-aligned -> the
   per-step act refresh lands at row 96 of the h2-tail chunk.
 - H=200 is split (128, 72): per-gate PSUM tiles are [128, 2, 512] (two
   banks; lanes 72..127 of the second bank hold junk that never escapes --
   the h/state writes slice [0:72]).

Only tactiles[0:10] is ever read (the model feeds back its own output after
the context window), so device I/O is tiny.  Host does all transposes.
"""

import os
import sys
import functools

sys.path.insert(0, "/opt/trn_rl_repo")

import numpy as np
import ml_dtypes

import concourse.bass as bass
from concourse import bacc
import concourse.tile as tile
from concourse import mybir
from concourse.bass_utils import run_bass_kernel_spmd

# model dims
T = 120
B = 4096
F = 48   # tactile feature size
A = 6    # action dim
H = 200  # LSTM hidden
CTX = 10
NSTEP = T - 1            # 119 scan steps
NOUT = NSTEP - (CTX - 1)  # 110 outputs
NCORES = 8
BL = B // NCORES         # 512 per-core batch
HA = 128                 # H chunk a
HB = H - HA              # H chunk b = 72
KB = 100                 # padded K of the h*b-only chunks
KD = 109                 # K of the h2b+act+state+ones chunk
KX = 100                 # K of the x1 chunk (48 feat + ones + zeros)
G4 = 4 * H               # 800 gate rows

COMPUTE_BF16 = True

LAST_RESULT = None  # BassKernelResults of the most recent run (for test.py)

Tanh = mybir.ActivationFunctionType.Tanh
Sigmoid = mybir.ActivationFunctionType.Sigmoid


def _dt():
    return mybir.dt.bfloat16 if COMPUTE_BF16 else mybir.dt.float32


def _npdt():
    return ml_dtypes.bfloat16 if COMPUTE_BF16 else np.float32


def _build_nc():
    nc = bacc.Bacc()
    dt = _dt()
    f32 = mybir.dt.float32

    # ---- DRAM parameters (per-core shards / replicated weights) ----
    # tact: [KX, CTX, BL]: rows 0..47 tactile features, row 48 ones, rest 0
    tact = nc.declare_dram_parameter("tact", [KX, CTX, BL], dt, isOutput=False)
    acts = nc.declare_dram_parameter("acts", [A, NSTEP, BL], dt, isOutput=False)
    # statzero: row 0 = ones, rows 8..13 = state, row 14 = ones, rest zeros
    statzero = nc.declare_dram_parameter("statzero", [64, BL], dt, isOutput=False)

    wshapes = {
        "w1ha": [HA, G4], "w1hb": [KB, G4], "w1x": [KX, G4],
        "w2h2a": [HA, G4], "w2td": [KD, G4], "w2h1a": [HA, G4],
        "w2h1b": [KB, G4],
        "w3ha": [HA, H], "w3td": [KD, H], "w3x": [KX, H],
        "w4a": [HA, F], "w4b": [96, F],
    }
    wd = {k: nc.declare_dram_parameter(k, s, dt, isOutput=False)
          for k, s in wshapes.items()}
    b4 = nc.declare_dram_parameter("b4", [F, 1], f32, isOutput=False)

    out = nc.declare_dram_parameter("out", [NOUT, F, BL], f32, isOutput=True)

    from contextlib import ExitStack

    with tile.TileContext(nc) as tc, ExitStack() as ctx:
        # ---- pools ----
        wpool = ctx.enter_context(tc.tile_pool(name="wpool", bufs=1))
        stp = ctx.enter_context(tc.tile_pool(name="stp", bufs=1))
        sp = ctx.enter_context(tc.tile_pool(name="sp", bufs=2))
        op = ctx.enter_context(tc.tile_pool(name="op", bufs=4))
        pp = ctx.enter_context(tc.tile_pool(name="pp", bufs=4, space="PSUM"))

        # ---- weights to SBUF (once) ----
        W = {}
        for k, s in wshapes.items():
            W[k] = wpool.tile(s, dt, name=k.upper())
            nc.sync.dma_start(out=W[k], in_=wd[k][:, :])
        B4 = wpool.tile([F, 1], f32, name="B4")
        nc.sync.dma_start(out=B4, in_=b4[:, :])

        # ---- persistent state / combined rhs K-chunk tiles ----
        # TA1/TA2: h1a/h2a [128].  TB1: [h1b(72); zeros(28)].
        # TD: [h2b(72); zeros(24); act(6)@96; state(6); ones(1)]
        # X1: [x1(48); ones(1)@48; zeros] ; TE/TF: o3 chunks for fc2
        TA1 = stp.tile([HA, BL], dt, name="TA1")
        TB1 = stp.tile([KB, BL], dt, name="TB1")
        TA2 = stp.tile([HA, BL], dt, name="TA2")
        TD = stp.tile([KD, BL], dt, name="TD")
        c1 = stp.tile([HA, 2, BL], f32, name="c1")
        c2 = stp.tile([HA, 2, BL], f32, name="c2")
        x1 = stp.tile([KX, BL], dt, name="x1")
        TE = stp.tile([HA, BL], dt, name="TE")
        TF = stp.tile([96, BL], dt, name="TF")
        TACT = stp.tile([KX, CTX, BL], dt, name="TACT")
        ACTS = stp.tile([A, NSTEP, BL], dt, name="ACTS")
        nc.sync.dma_start(out=TACT, in_=tact[:, :, :])
        nc.sync.dma_start(out=ACTS, in_=acts[:, :, :])
        # x1 rows 48..99 <- statzero rows 0..51 (row 0 = ones)
        nc.sync.dma_start(out=x1[F:KX, :], in_=statzero[0:KX - F, :])
        # TD rows 102..108 <- statzero rows 8..14 (state + ones)
        nc.sync.dma_start(out=TD[96 + A:KD, :], in_=statzero[8:8 + A + 1, :])
        nc.vector.memset(TA1, 0.0)
        nc.vector.memset(TB1, 0.0)
        nc.vector.memset(TA2, 0.0)
        nc.vector.memset(TD[0:96, :], 0.0)
        nc.vector.memset(TF, 0.0)
        nc.vector.memset(c1, 0.0)
        nc.vector.memset(c2, 0.0)

        # gate column layout (permuted rows [i f o g], chunks a=128/b=72)
        GBASE = {"i": 0, "f": 200, "o": 400, "g": 600}

        def lstm_gates(kchunks, tag):
            """kchunks: list of (weight_key, rhs) accumulated in order.
            Per gate one [128, 2, BL] PSUM tile: bank0 = a-chunk (M=128),
            bank1 = b-chunk (M=72, lanes 72..127 junk)."""
            P = {}
            for gate in ("g", "i", "f", "o"):
                gp = pp.tile([HA, 2, BL], f32, name=f"P{gate}_{tag}", tag="g")
                for m, (mo, mn) in enumerate(((0, HA), (HA, H))):
                    ps = gp[0:mn - mo, m, :]
                    cols = slice(GBASE[gate] + mo, GBASE[gate] + mn)
                    last = len(kchunks) - 1
                    for j, (wk, rhs) in enumerate(kchunks):
                        nc.tensor.matmul(ps, W[wk][:, cols], rhs,
                                         start=(j == 0), stop=(j == last))
                P[gate] = gp
            return P

        def lstm_cell(P, c, ha, hb, tag):
            """update c (f32 [128,2,BL]) and h (ha [128,BL], hb [72,BL])"""
            gt = sp.tile([HA, 2, BL], dt, name=f"gt{tag}", tag="gt")
            sgi = sp.tile([HA, 2, BL], dt, name=f"sgi{tag}", tag="sgi")
            sgf = sp.tile([HA, 2, BL], dt, name=f"sgf{tag}", tag="sgf")
            sgo = sp.tile([HA, 2, BL], dt, name=f"sgo{tag}", tag="sgo")
            nc.scalar.activation(gt, P["g"], Tanh)
            nc.scalar.activation(sgi, P["i"], Sigmoid)
            nc.scalar.activation(sgf, P["f"], Sigmoid)
            nc.scalar.activation(sgo, P["o"], Sigmoid)
            ig = sp.tile([HA, 2, BL], dt, name=f"ig{tag}", tag="ig")
            fm = sp.tile([HA, 2, BL], f32, name=f"fm{tag}", tag="fm")
            nc.vector.tensor_mul(ig, sgi, gt)
            nc.vector.tensor_mul(fm, sgf, c)
            nc.vector.tensor_add(c, fm, ig)
            tch = sp.tile([HA, 2, BL], dt, name=f"tch{tag}", tag="tch")
            nc.scalar.activation(tch, c, Tanh)
            nc.vector.tensor_mul(ha, sgo[:, 0, :], tch[:, 0, :])
            nc.vector.tensor_mul(hb, sgo[0:HB, 1, :], tch[0:HB, 1, :])

        for t in range(NSTEP):
            x1_rhs = TACT[:, t, :] if t < CTX else x1
            # refresh act rows of TD (aligned offset 96)
            nc.vector.tensor_copy(TD[96:96 + A, :], ACTS[:, t, :])

            # ---- LSTM1: gates = W1h·h1 + W1x·[x1;1] ----
            P1 = lstm_gates([("w1ha", TA1), ("w1hb", TB1), ("w1x", x1_rhs)],
                            f"1_{t}")
            lstm_cell(P1, c1, TA1, TB1[0:HB, :], f"1_{t}")

            # ---- LSTM2: gates = W2h·h2 + W2as·[act;state;1] + W2x·h1 ----
            P2 = lstm_gates([("w2h2a", TA2), ("w2td", TD),
                             ("w2h1a", TA1), ("w2h1b", TB1)], f"2_{t}")
            lstm_cell(P2, c2, TA2, TD[0:HB, :], f"2_{t}")

            # ---- MLP head (only needed from t = CTX-1 on) ----
            if t >= CTX - 1:
                fcp = pp.tile([HA, 2, BL], f32, name=f"fcp_{t}", tag="g")
                for m, (mo, mn) in enumerate(((0, HA), (HA, H))):
                    ps = fcp[0:mn - mo, m, :]
                    cols = slice(mo, mn)
                    nc.tensor.matmul(ps, W["w3x"][:, cols], x1_rhs,
                                     start=True, stop=False)
                    nc.tensor.matmul(ps, W["w3ha"][:, cols], TA2,
                                     start=False, stop=False)
                    nc.tensor.matmul(ps, W["w3td"][:, cols], TD,
                                     start=False, stop=True)
                nc.scalar.activation(TE, fcp[:, 0, :], Tanh)
                nc.scalar.activation(TF[0:HB, :], fcp[0:HB, 1, :], Tanh)
                f2p = pp.tile([F, BL], f32, name=f"f2p_{t}", tag="g")
                nc.tensor.matmul(f2p, W["w4a"], TE, start=True, stop=False)
                nc.tensor.matmul(f2p, W["w4b"], TF, start=False, stop=True)
                stg = op.tile([F, BL], f32, name=f"stg_{t}", tag="stg")
                if t < NSTEP - 1:
                    # feedback: next step's input (fc2 bias via ACT bias)
                    nc.scalar.activation(x1[0:F, :], f2p, Tanh, bias=B4)
                    nc.vector.tensor_copy(stg, x1[0:F, :])
                else:
                    nc.scalar.activation(stg, f2p, Tanh, bias=B4)
                nc.gpsimd.dma_start(out=out[t - (CTX - 1)], in_=stg)

    nc.finalize()
    return nc


@functools.lru_cache(maxsize=1)
def _get_nc():
    return _build_nc()


def _prep_weights(W_ih1, W_hh1, b_ih1, b_hh1, W_ih2, W_hh2, b_ih2, b_hh2,
                  fc1_w, fc1_b, fc2_w, fc2_b):
    # gate rows reordered [i, f, o, g]
    perm = np.concatenate([np.arange(0, 200), np.arange(200, 400),
                           np.arange(600, 800), np.arange(400, 600)])
    W1p = np.asarray(W_ih1)[perm]          # [800, 48]
    W1hp = np.asarray(W_hh1)[perm]         # [800, 200]
    b1p = (np.asarray(b_ih1) + np.asarray(b_hh1))[perm]
    W2p = np.asarray(W_ih2)[perm]          # [800, 248]
    W2hp = np.asarray(W_hh2)[perm]         # [800, 200]
    b2p = (np.asarray(b_ih2) + np.asarray(b_hh2))[perm]
    Wt = W2p[:, 200:248]
    W2eff = Wt[:, 0:12] + Wt[:, 12:24] + Wt[:, 24:36] + Wt[:, 36:48]  # [800,12]
    fc1_w = np.asarray(fc1_w); fc1_b = np.asarray(fc1_b)
    fc2_w = np.asarray(fc2_w); fc2_b = np.asarray(fc2_b)
    npdt = _npdt()

    def c(x):
        return np.ascontiguousarray(x).astype(npdt)

    def pad_to(x, k):
        z = np.zeros((k, x.shape[1]), np.float32)
        z[:x.shape[0]] = x
        return z

    def td_weights(w_hb_T, w_as_T, bias):
        z = np.zeros((KD, w_hb_T.shape[1]), np.float32)
        z[0:HB] = w_hb_T
        z[96:108] = w_as_T
        z[108] = bias
        return z

    return {
        "w1ha": c(W1hp[:, 0:HA].T),
        "w1hb": c(pad_to(W1hp[:, HA:H].T, KB)),
        "w1x": c(pad_to(np.concatenate([W1p.T, b1p[None, :]], 0), KX)),
        "w2h2a": c(W2hp[:, 0:HA].T),
        "w2td": c(td_weights(W2hp[:, HA:H].T, W2eff.T, b2p)),
        "w2h1a": c(W2p[:, 0:HA].T),
        "w2h1b": c(pad_to(W2p[:, HA:H].T, KB)),
        "w3ha": c(fc1_w[:, 0:HA].T),
        "w3td": c(pad_to(fc1_w[:, HA:H].T, KD)),
        "w3x": c(pad_to(np.concatenate([fc1_w[:, 200:248].T,
                                        fc1_b[None, :]], 0), KX)),
        "w4a": c(fc2_w[:, 0:HA].T),
        "w4b": c(pad_to(fc2_w[:, HA:H].T, 96)),
        "b4": np.ascontiguousarray(fc2_b[:, None]).astype(np.float32),
    }


def kernel(tactiles, actions, W_ih1, W_hh1, b_ih1, b_hh1,
           W_ih2, W_hh2, b_ih2, b_hh2, fc1_w, fc1_b, fc2_w, fc2_b):
    global LAST_RESULT
    npdt = _npdt()
    tactiles = np.asarray(tactiles)
    actions = np.asarray(actions)

    wmap = _prep_weights(W_ih1, W_hh1, b_ih1, b_hh1, W_ih2, W_hh2, b_ih2, b_hh2,
                         fc1_w, fc1_b, fc2_w, fc2_b)

    in_maps = []
    for i in range(NCORES):
        s = slice(i * BL, (i + 1) * BL)
        tt = np.zeros((KX, CTX, BL), np.float32)
        tt[0:F] = tactiles[0:CTX, s, :].transpose(2, 0, 1)
        tt[F] = 1.0
        acts_T = np.ascontiguousarray(
            actions[1:T, s, :].transpose(2, 0, 1)).astype(npdt)   # [6,119,BL]
        sz = np.zeros((64, BL), np.float32)
        sz[0] = 1.0                      # x1 ones row
        sz[8:8 + A] = actions[0, s, :].T  # state rows
        sz[8 + A] = 1.0                  # TD ones row
        m = {"tact": tt.astype(npdt), "acts": acts_T,
             "statzero": sz.astype(npdt)}
        m.update(wmap)
        in_maps.append(m)

    nc = _get_nc()
    res = run_bass_kernel_spmd(nc, in_maps, core_ids=list(range(NCORES)))
    LAST_RESULT = res

    outs = [np.asarray(r["out"], dtype=np.float32) for r in res.results]
    full = np.concatenate([o.transpose(0, 2, 1) for o in outs], axis=1)
    return np.ascontiguousarray(full)


# revision 13
# speedup vs baseline: 1.2009x; 1.1062x over previous
"""Trainium2 Bass kernel for the ACTP 2-layer-LSTM + MLP rollout model.

Strategy: pure data parallel across 8 NeuronCores (batch 4096 -> 512/core),
weights replicated.  All on-chip tensors are feature-major [feat, batch] so
the time recurrence needs no transposes: matmuls are out[M,N] = W_T[K,M].T @
x[K,N] with the batch as the moving free dim (N=512).  Every concat in the
model becomes extra K-chunk matmuls accumulating into the same PSUM bank,
biases ride in "ones-row" K-chunks or the activation bias operand, and the
tiled(act,state) input of LSTM2 collapses algebraically into rows of the
h2-tail K-chunk.

Layout rules discovered on hardware:
 - matmuls with K <= ~64 anywhere in the stream permanently block the PE HAM
   clock-gate from reaching 2.4 GHz -> every K-chunk is padded to >= 96 rows
   (zero weight rows; rhs pad rows zeroed so 0*0 can't make NaN).
 - compute-engine writes at a partition offset must be 32-aligned -> the
   per-step act refresh lands at row 96 of the h2-tail chunk.
 - H=200 is split (128, 72): per-gate PSUM tiles are [128, 2, 512] (two
   banks; lanes 72..127 of the second bank hold junk that never escapes --
   the h/state writes slice [0:72]).

Only tactiles[0:10] is ever read (the model feeds back its own output after
the context window), so device I/O is tiny.  Host does all transposes.
"""

import os
import sys
import functools

sys.path.insert(0, "/opt/trn_rl_repo")

import numpy as np
import ml_dtypes

import concourse.bass as bass
from concourse import bacc
import concourse.tile as tile
from concourse import mybir
from concourse.bass_utils import run_bass_kernel_spmd

# model dims
T = 120
B = 4096
F = 48   # tactile feature size
A = 6    # action dim
H = 200  # LSTM hidden
CTX = 10
NSTEP = T - 1            # 119 scan steps
NOUT = NSTEP - (CTX - 1)  # 110 outputs
NCORES = 8
BL = B // NCORES         # 512 per-core batch
BH = BL // 2             # interleaved independent half-batch
HA = 128                 # H chunk a
HB = H - HA              # H chunk b = 72
KB = 100                 # padded K of the h*b-only chunks
KD = 109                 # K of the h2b+act+state+ones chunk
KX = 100                 # K of the x1 chunk (48 feat + ones + zeros)
G4 = 4 * H               # 800 gate rows

COMPUTE_BF16 = True

LAST_RESULT = None  # BassKernelResults of the most recent run (for test.py)

Tanh = mybir.ActivationFunctionType.Tanh
Sigmoid = mybir.ActivationFunctionType.Sigmoid


def _dt():
    return mybir.dt.bfloat16 if COMPUTE_BF16 else mybir.dt.float32


def _npdt():
    return ml_dtypes.bfloat16 if COMPUTE_BF16 else np.float32


def _build_nc():
    nc = bacc.Bacc()
    dt = _dt()
    f32 = mybir.dt.float32

    # ---- DRAM parameters (per-core shards / replicated weights) ----
    # tact: [KX, CTX, BL]: rows 0..47 tactile features, row 48 ones, rest 0
    tact = nc.declare_dram_parameter("tact", [KX, CTX, BL], dt, isOutput=False)
    acts = nc.declare_dram_parameter("acts", [A, NSTEP, BL], dt, isOutput=False)
    # statzero: row 0 = ones, rows 8..13 = state, row 14 = ones, rest zeros
    statzero = nc.declare_dram_parameter("statzero", [64, BL], dt, isOutput=False)

    wshapes = {
        "w1ha": [HA, G4], "w1hb": [KB, G4], "w1x": [KX, G4],
        "w2h2a": [HA, G4], "w2td": [KD, G4], "w2h1a": [HA, G4],
        "w2h1b": [KB, G4],
        "w3ha": [HA, H], "w3td": [KD, H], "w3x": [KX, H],
        "w4a": [HA, F], "w4b": [96, F],
    }
    wd = {k: nc.declare_dram_parameter(k, s, dt, isOutput=False)
          for k, s in wshapes.items()}
    b4 = nc.declare_dram_parameter("b4", [F, 1], f32, isOutput=False)

    out = nc.declare_dram_parameter("out", [NOUT, F, BL], f32, isOutput=True)

    from contextlib import ExitStack

    with tile.TileContext(nc) as tc, ExitStack() as ctx:
        # ---- pools ----
        wpool = ctx.enter_context(tc.tile_pool(name="wpool", bufs=1))
        stp = ctx.enter_context(tc.tile_pool(name="stp", bufs=1))
        sp = ctx.enter_context(tc.tile_pool(name="sp", bufs=2))
        op = ctx.enter_context(tc.tile_pool(name="op", bufs=4))
        pp = ctx.enter_context(tc.tile_pool(name="pp", bufs=8, space="PSUM"))

        # ---- weights to SBUF (once) ----
        W = {}
        for k, s in wshapes.items():
            W[k] = wpool.tile(s, dt, name=k.upper())
            nc.sync.dma_start(out=W[k], in_=wd[k][:, :])
        B4 = wpool.tile([F, 1], f32, name="B4")
        nc.sync.dma_start(out=B4, in_=b4[:, :])

        # ---- persistent state / combined rhs K-chunk tiles ----
        # TA1/TA2: h1a/h2a [128].  TB1: [h1b(72); zeros(28)].
        # TD: [h2b(72); zeros(24); act(6)@96; state(6); ones(1)]
        # X1: [x1(48); ones(1)@48; zeros] ; TE/TF: o3 chunks for fc2
        TACT = stp.tile([KX, CTX, BL], dt, name="TACT")
        ACTS = stp.tile([A, NSTEP, BL], dt, name="ACTS")
        nc.sync.dma_start(out=TACT, in_=tact[:, :, :])
        nc.sync.dma_start(out=ACTS, in_=acts[:, :, :])
        halves = []
        for hx in range(2):
            cs = slice(hx * BH, (hx + 1) * BH)
            hh = {}
            hh["cs"] = cs
            hh["TA1"] = stp.tile([HA, BH], dt, name=f"TA1_{hx}")
            hh["TB1"] = stp.tile([KB, BH], dt, name=f"TB1_{hx}")
            hh["TA2"] = stp.tile([HA, BH], dt, name=f"TA2_{hx}")
            hh["TD"] = stp.tile([KD, BH], dt, name=f"TD_{hx}")
            hh["c1"] = stp.tile([HA, 2, BH], f32, name=f"c1_{hx}")
            hh["c2"] = stp.tile([HA, 2, BH], f32, name=f"c2_{hx}")
            hh["x1"] = stp.tile([KX, BH], dt, name=f"x1_{hx}")
            hh["TE"] = stp.tile([HA, BH], dt, name=f"TE_{hx}")
            hh["TF"] = stp.tile([96, BH], dt, name=f"TF_{hx}")
            nc.sync.dma_start(out=hh["x1"][F:KX, :], in_=statzero[0:KX - F, cs])
            nc.sync.dma_start(out=hh["TD"][96 + A:KD, :],
                              in_=statzero[8:8 + A + 1, cs])
            nc.vector.memset(hh["TA1"], 0.0)
            nc.vector.memset(hh["TB1"], 0.0)
            nc.vector.memset(hh["TA2"], 0.0)
            nc.vector.memset(hh["TD"][0:96, :], 0.0)
            nc.vector.memset(hh["TF"], 0.0)
            nc.vector.memset(hh["c1"], 0.0)
            nc.vector.memset(hh["c2"], 0.0)
            halves.append(hh)

        # gate column layout (permuted rows [i f o g], chunks a=128/b=72)
        GBASE = {"i": 0, "f": 200, "o": 400, "g": 600}

        def lstm_gates(kchunks, tag):
            """kchunks: list of (weight_key, rhs) accumulated in order.
            Per gate one [128, 2, BH] PSUM tile (one bank): slot0 = a-chunk
            (M=128), slot1 = b-chunk (M=72, lanes 72..127 junk)."""
            P = {}
            for gate in ("g", "i", "f", "o"):
                gp = pp.tile([HA, 2, BH], f32, name=f"P{gate}_{tag}", tag="g")
                for m, (mo, mn) in enumerate(((0, HA), (HA, H))):
                    ps = gp[0:mn - mo, m, :]
                    cols = slice(GBASE[gate] + mo, GBASE[gate] + mn)
                    last = len(kchunks) - 1
                    for j, (wk, rhs) in enumerate(kchunks):
                        nc.tensor.matmul(ps, W[wk][:, cols], rhs,
                                         start=(j == 0), stop=(j == last))
                P[gate] = gp
            return P

        def lstm_cell(P, c, ha, hb, tag):
            """update c (f32 [128,2,BH]) and h (ha [128,BH], hb [72,BH])"""
            gt = sp.tile([HA, 2, BH], dt, name=f"gt{tag}", tag="gt")
            sgi = sp.tile([HA, 2, BH], dt, name=f"sgi{tag}", tag="sgi")
            sgf = sp.tile([HA, 2, BH], dt, name=f"sgf{tag}", tag="sgf")
            sgo = sp.tile([HA, 2, BH], dt, name=f"sgo{tag}", tag="sgo")
            nc.scalar.activation(gt, P["g"], Tanh)
            nc.scalar.activation(sgi, P["i"], Sigmoid)
            nc.scalar.activation(sgf, P["f"], Sigmoid)
            nc.scalar.activation(sgo, P["o"], Sigmoid)
            ig = sp.tile([HA, 2, BH], dt, name=f"ig{tag}", tag="ig")
            fm = sp.tile([HA, 2, BH], f32, name=f"fm{tag}", tag="fm")
            nc.vector.tensor_mul(ig, sgi, gt)
            nc.vector.tensor_mul(fm, sgf, c)
            nc.vector.tensor_add(c, fm, ig)
            tch = sp.tile([HA, 2, BH], dt, name=f"tch{tag}", tag="tch")
            nc.scalar.activation(tch, c, Tanh)
            nc.vector.tensor_mul(ha, sgo[:, 0, :], tch[:, 0, :])
            nc.vector.tensor_mul(hb, sgo[0:HB, 1, :], tch[0:HB, 1, :])

        def emit_lstm1(hh, t, hx):
            x1_rhs = TACT[:, t, hh["cs"]] if t < CTX else hh["x1"]
            nc.vector.tensor_copy(hh["TD"][96:96 + A, :],
                                  ACTS[:, t, hh["cs"]])
            P1 = lstm_gates([("w1ha", hh["TA1"]), ("w1hb", hh["TB1"]),
                             ("w1x", x1_rhs)], f"1_{t}_{hx}")
            lstm_cell(P1, hh["c1"], hh["TA1"], hh["TB1"][0:HB, :],
                      f"1_{t}_{hx}")

        def emit_lstm2(hh, t, hx):
            P2 = lstm_gates([("w2h2a", hh["TA2"]), ("w2td", hh["TD"]),
                             ("w2h1a", hh["TA1"]), ("w2h1b", hh["TB1"])],
                            f"2_{t}_{hx}")
            lstm_cell(P2, hh["c2"], hh["TA2"], hh["TD"][0:HB, :],
                      f"2_{t}_{hx}")

        def emit_fc(hh, t, hx):
            x1_rhs = TACT[:, t, hh["cs"]] if t < CTX else hh["x1"]
            fcp = pp.tile([HA, 2, BH], f32, name=f"fcp_{t}_{hx}", tag="g")
            for m, (mo, mn) in enumerate(((0, HA), (HA, H))):
                ps = fcp[0:mn - mo, m, :]
                cols = slice(mo, mn)
                nc.tensor.matmul(ps, W["w3x"][:, cols], x1_rhs,
                                 start=True, stop=False)
                nc.tensor.matmul(ps, W["w3ha"][:, cols], hh["TA2"],
                                 start=False, stop=False)
                nc.tensor.matmul(ps, W["w3td"][:, cols], hh["TD"],
                                 start=False, stop=True)
            nc.scalar.activation(hh["TE"], fcp[:, 0, :], Tanh)
            nc.scalar.activation(hh["TF"][0:HB, :], fcp[0:HB, 1, :], Tanh)
            f2p = pp.tile([F, BH], f32, name=f"f2p_{t}_{hx}", tag="g")
            nc.tensor.matmul(f2p, W["w4a"], hh["TE"], start=True, stop=False)
            nc.tensor.matmul(f2p, W["w4b"], hh["TF"], start=False, stop=True)
            stg = op.tile([F, BH], f32, name=f"stg_{t}_{hx}", tag="stg")
            if t < NSTEP - 1:
                nc.scalar.activation(hh["x1"][0:F, :], f2p, Tanh, bias=B4)
                nc.vector.tensor_copy(stg, hh["x1"][0:F, :])
            else:
                nc.scalar.activation(stg, f2p, Tanh, bias=B4)
            nc.gpsimd.dma_start(out=out[t - (CTX - 1), :, hh["cs"]], in_=stg)

        X, Y = halves
        # interleave the two independent half-batch recurrences: while one
        # half runs its elementwise chain the other half's matmuls keep the
        # PE busy (and the HAM clock-gate warm)
        for t in range(NSTEP):
            emit_lstm1(X, t, 0)
            emit_lstm1(Y, t, 1)
            emit_lstm2(X, t, 0)
            emit_lstm2(Y, t, 1)
            if t >= CTX - 1:
                emit_fc(X, t, 0)
                emit_fc(Y, t, 1)

    nc.finalize()
    return nc


@functools.lru_cache(maxsize=1)
def _get_nc():
    return _build_nc()


def _prep_weights(W_ih1, W_hh1, b_ih1, b_hh1, W_ih2, W_hh2, b_ih2, b_hh2,
                  fc1_w, fc1_b, fc2_w, fc2_b):
    # gate rows reordered [i, f, o, g]
    perm = np.concatenate([np.arange(0, 200), np.arange(200, 400),
                           np.arange(600, 800), np.arange(400, 600)])
    W1p = np.asarray(W_ih1)[perm]          # [800, 48]
    W1hp = np.asarray(W_hh1)[perm]         # [800, 200]
    b1p = (np.asarray(b_ih1) + np.asarray(b_hh1))[perm]
    W2p = np.asarray(W_ih2)[perm]          # [800, 248]
    W2hp = np.asarray(W_hh2)[perm]         # [800, 200]
    b2p = (np.asarray(b_ih2) + np.asarray(b_hh2))[perm]
    Wt = W2p[:, 200:248]
    W2eff = Wt[:, 0:12] + Wt[:, 12:24] + Wt[:, 24:36] + Wt[:, 36:48]  # [800,12]
    fc1_w = np.asarray(fc1_w); fc1_b = np.asarray(fc1_b)
    fc2_w = np.asarray(fc2_w); fc2_b = np.asarray(fc2_b)
    npdt = _npdt()

    def c(x):
        return np.ascontiguousarray(x).astype(npdt)

    def pad_to(x, k):
        z = np.zeros((k, x.shape[1]), np.float32)
        z[:x.shape[0]] = x
        return z

    def td_weights(w_hb_T, w_as_T, bias):
        z = np.zeros((KD, w_hb_T.shape[1]), np.float32)
        z[0:HB] = w_hb_T
        z[96:108] = w_as_T
        z[108] = bias
        return z

    return {
        "w1ha": c(W1hp[:, 0:HA].T),
        "w1hb": c(pad_to(W1hp[:, HA:H].T, KB)),
        "w1x": c(pad_to(np.concatenate([W1p.T, b1p[None, :]], 0), KX)),
        "w2h2a": c(W2hp[:, 0:HA].T),
        "w2td": c(td_weights(W2hp[:, HA:H].T, W2eff.T, b2p)),
        "w2h1a": c(W2p[:, 0:HA].T),
        "w2h1b": c(pad_to(W2p[:, HA:H].T, KB)),
        "w3ha": c(fc1_w[:, 0:HA].T),
        "w3td": c(pad_to(fc1_w[:, HA:H].T, KD)),
        "w3x": c(pad_to(np.concatenate([fc1_w[:, 200:248].T,
                                        fc1_b[None, :]], 0), KX)),
        "w4a": c(fc2_w[:, 0:HA].T),
        "w4b": c(pad_to(fc2_w[:, HA:H].T, 96)),
        "b4": np.ascontiguousarray(fc2_b[:, None]).astype(np.float32),
    }


def kernel(tactiles, actions, W_ih1, W_hh1, b_ih1, b_hh1,
           W_ih2, W_hh2, b_ih2, b_hh2, fc1_w, fc1_b, fc2_w, fc2_b):
    global LAST_RESULT
    npdt = _npdt()
    tactiles = np.asarray(tactiles)
    actions = np.asarray(actions)

    wmap = _prep_weights(W_ih1, W_hh1, b_ih1, b_hh1, W_ih2, W_hh2, b_ih2, b_hh2,
                         fc1_w, fc1_b, fc2_w, fc2_b)

    in_maps = []
    for i in range(NCORES):
        s = slice(i * BL, (i + 1) * BL)
        tt = np.zeros((KX, CTX, BL), np.float32)
        tt[0:F] = tactiles[0:CTX, s, :].transpose(2, 0, 1)
        tt[F] = 1.0
        acts_T = np.ascontiguousarray(
            actions[1:T, s, :].transpose(2, 0, 1)).astype(npdt)   # [6,119,BL]
        sz = np.zeros((64, BL), np.float32)
        sz[0] = 1.0                      # x1 ones row
        sz[8:8 + A] = actions[0, s, :].T  # state rows
        sz[8 + A] = 1.0                  # TD ones row
        m = {"tact": tt.astype(npdt), "acts": acts_T,
             "statzero": sz.astype(npdt)}
        m.update(wmap)
        in_maps.append(m)

    nc = _get_nc()
    res = run_bass_kernel_spmd(nc, in_maps, core_ids=list(range(NCORES)))
    LAST_RESULT = res

    outs = [np.asarray(r["out"], dtype=np.float32) for r in res.results]
    full = np.concatenate([o.transpose(0, 2, 1) for o in outs], axis=1)
    return np.ascontiguousarray(full)


# revision 14
# speedup vs baseline: 1.6730x; 1.3931x over previous
"""Trainium2 Bass kernel for the ACTP 2-layer-LSTM + MLP rollout model.

Strategy: pure data parallel across 8 NeuronCores (batch 4096 -> 512/core),
weights replicated.  All on-chip tensors are feature-major [feat, batch] so
the time recurrence needs no transposes: matmuls are out[M,N] = W_T[K,M].T @
x[K,N] with the batch as the moving free dim (N=512).  Every concat in the
model becomes extra K-chunk matmuls accumulating into the same PSUM bank,
biases ride in "ones-row" K-chunks or the activation bias operand, and the
tiled(act,state) input of LSTM2 collapses algebraically into rows of the
h2-tail K-chunk.

Layout rules discovered on hardware:
 - matmuls with K <= ~64 anywhere in the stream permanently block the PE HAM
   clock-gate from reaching 2.4 GHz -> every K-chunk is padded to >= 96 rows
   (zero weight rows; rhs pad rows zeroed so 0*0 can't make NaN).
 - compute-engine writes at a partition offset must be 32-aligned -> the
   per-step act refresh lands at row 96 of the h2-tail chunk.
 - H=200 is split (128, 72): per-gate PSUM tiles are [128, 2, 512] (two
   banks; lanes 72..127 of the second bank hold junk that never escapes --
   the h/state writes slice [0:72]).

Only tactiles[0:10] is ever read (the model feeds back its own output after
the context window), so device I/O is tiny.  Host does all transposes.
"""

import os
import sys
import functools

sys.path.insert(0, "/opt/trn_rl_repo")

import numpy as np
import ml_dtypes

import concourse.bass as bass
from concourse import bacc
import concourse.tile as tile
from concourse import mybir
from concourse.bass_utils import run_bass_kernel_spmd

# model dims
T = 120
B = 4096
F = 48   # tactile feature size
A = 6    # action dim
H = 200  # LSTM hidden
CTX = 10
NSTEP = T - 1            # 119 scan steps
NOUT = NSTEP - (CTX - 1)  # 110 outputs
NCORES = 8
BL = B // NCORES         # 512 per-core batch
BH = BL // 2             # interleaved independent half-batch
HA = 128                 # H chunk a
HB = H - HA              # H chunk b = 72
KB = 100                 # padded K of the h*b-only chunks
KD = 109                 # K of the h2b+act+state+ones chunk
KX = 100                 # K of the x1 chunk (48 feat + ones + zeros)
G4 = 4 * H               # 800 gate rows

COMPUTE_BF16 = True

LAST_RESULT = None  # BassKernelResults of the most recent run (for test.py)

Tanh = mybir.ActivationFunctionType.Tanh
Sigmoid = mybir.ActivationFunctionType.Sigmoid


def _dt():
    return mybir.dt.bfloat16 if COMPUTE_BF16 else mybir.dt.float32


def _npdt():
    return ml_dtypes.bfloat16 if COMPUTE_BF16 else np.float32


def _build_nc():
    nc = bacc.Bacc()
    dt = _dt()
    f32 = mybir.dt.float32

    # ---- DRAM parameters (per-core shards / replicated weights) ----
    # tact: [KX, CTX, BL]: rows 0..47 tactile features, row 48 ones, rest 0
    tact = nc.declare_dram_parameter("tact", [KX, CTX, BL], dt, isOutput=False)
    acts = nc.declare_dram_parameter("acts", [A, NSTEP, BL], dt, isOutput=False)
    # statzero: row 0 = ones, rows 8..13 = state, row 14 = ones, rest zeros
    statzero = nc.declare_dram_parameter("statzero", [64, BL], dt, isOutput=False)

    wshapes = {
        "w1ha": [HA, G4], "w1hb": [KB, G4], "w1x": [KX, G4],
        "w2h2a": [HA, G4], "w2td": [KD, G4], "w2h1a": [HA, G4],
        "w2h1b": [KB, G4],
        "w3ha": [HA, H], "w3td": [KD, H], "w3x": [KX, H],
        "w4a": [HA, F], "w4b": [96, F],
    }
    wd = {k: nc.declare_dram_parameter(k, s, dt, isOutput=False)
          for k, s in wshapes.items()}
    b4 = nc.declare_dram_parameter("b4", [F, 1], f32, isOutput=False)

    out = nc.declare_dram_parameter("out", [NOUT, F, BL], f32, isOutput=True)

    from contextlib import ExitStack

    with tile.TileContext(nc) as tc, ExitStack() as ctx:
        # ---- pools ----
        wpool = ctx.enter_context(tc.tile_pool(name="wpool", bufs=1))
        stp = ctx.enter_context(tc.tile_pool(name="stp", bufs=1))
        sp = ctx.enter_context(tc.tile_pool(name="sp", bufs=2))
        op = ctx.enter_context(tc.tile_pool(name="op", bufs=4))
        pp = ctx.enter_context(tc.tile_pool(name="pp", bufs=8, space="PSUM"))

        # ---- weights to SBUF (once) ----
        W = {}
        for k, s in wshapes.items():
            W[k] = wpool.tile(s, dt, name=k.upper())
            nc.sync.dma_start(out=W[k], in_=wd[k][:, :])
        B4 = wpool.tile([F, 1], f32, name="B4")
        nc.sync.dma_start(out=B4, in_=b4[:, :])

        # ---- persistent state / combined rhs K-chunk tiles ----
        # TA1/TA2: h1a/h2a [128].  TB1: [h1b(72); zeros(28)].
        # TD: [h2b(72); zeros(24); act(6)@96; state(6); ones(1)]
        # X1: [x1(48); ones(1)@48; zeros] ; TE/TF: o3 chunks for fc2
        TACT = stp.tile([KX, CTX, BL], dt, name="TACT")
        ACTS = stp.tile([A, NSTEP, BL], dt, name="ACTS")
        nc.sync.dma_start(out=TACT, in_=tact[:, :, :])
        nc.sync.dma_start(out=ACTS, in_=acts[:, :, :])
        halves = []
        for hx in range(2):
            cs = slice(hx * BH, (hx + 1) * BH)
            hh = {}
            hh["cs"] = cs
            hh["TA1"] = stp.tile([HA, BH], dt, name=f"TA1_{hx}")
            hh["TB1"] = stp.tile([KB, BH], dt, name=f"TB1_{hx}")
            hh["TA2"] = stp.tile([HA, BH], dt, name=f"TA2_{hx}")
            hh["TD"] = stp.tile([KD, BH], dt, name=f"TD_{hx}")
            hh["c1"] = stp.tile([HA, 2, BH], f32, name=f"c1_{hx}")
            hh["c2"] = stp.tile([HA, 2, BH], f32, name=f"c2_{hx}")
            hh["x1"] = stp.tile([KX, BH], dt, name=f"x1_{hx}")
            hh["TE"] = stp.tile([HA, BH], dt, name=f"TE_{hx}")
            hh["TF"] = stp.tile([96, BH], dt, name=f"TF_{hx}")
            nc.sync.dma_start(out=hh["x1"][F:KX, :], in_=statzero[0:KX - F, cs])
            nc.sync.dma_start(out=hh["TD"][96 + A:KD, :],
                              in_=statzero[8:8 + A + 1, cs])
            nc.vector.memset(hh["TA1"], 0.0)
            nc.vector.memset(hh["TB1"], 0.0)
            nc.vector.memset(hh["TA2"], 0.0)
            nc.vector.memset(hh["TD"][0:96, :], 0.0)
            nc.vector.memset(hh["TF"], 0.0)
            nc.vector.memset(hh["c1"], 0.0)
            nc.vector.memset(hh["c2"], 0.0)
            halves.append(hh)

        # gate column layout (permuted rows [i f o g], chunks a=128/b=72)
        GBASE = {"i": 0, "f": 200, "o": 400, "g": 600}

        def lstm_gates(kchunks, tag):
            """kchunks: list of (weight_key, rhs) accumulated in order.
            Per gate one [128, 2, BH] PSUM tile (one bank): slot0 = a-chunk
            (M=128), slot1 = b-chunk (M=72, lanes 72..127 junk)."""
            P = {}
            for gate in ("g", "i", "f", "o"):
                gp = pp.tile([HA, 2, BH], f32, name=f"P{gate}_{tag}", tag="g")
                for m, (mo, mn) in enumerate(((0, HA), (HA, H))):
                    ps = gp[0:mn - mo, m, :]
                    cols = slice(GBASE[gate] + mo, GBASE[gate] + mn)
                    last = len(kchunks) - 1
                    for j, (wk, rhs) in enumerate(kchunks):
                        nc.tensor.matmul(ps, W[wk][:, cols], rhs,
                                         start=(j == 0), stop=(j == last))
                P[gate] = gp
            return P

        def lstm_cell(P, c, ha, hb, tag):
            """update c (f32 [128,2,BH]) and h (ha [128,BH], hb [72,BH])"""
            gt = sp.tile([HA, 2, BH], dt, name=f"gt{tag}", tag="gt")
            sgi = sp.tile([HA, 2, BH], dt, name=f"sgi{tag}", tag="sgi")
            sgf = sp.tile([HA, 2, BH], dt, name=f"sgf{tag}", tag="sgf")
            sgo = sp.tile([HA, 2, BH], dt, name=f"sgo{tag}", tag="sgo")
            nc.scalar.activation(gt, P["g"], Tanh)
            nc.scalar.activation(sgi, P["i"], Sigmoid)
            nc.scalar.activation(sgf, P["f"], Sigmoid)
            nc.scalar.activation(sgo, P["o"], Sigmoid)
            ig = sp.tile([HA, 2, BH], dt, name=f"ig{tag}", tag="ig")
            fm = sp.tile([HA, 2, BH], f32, name=f"fm{tag}", tag="fm")
            nc.vector.tensor_mul(ig, sgi, gt)
            nc.vector.tensor_mul(fm, sgf, c)
            nc.vector.tensor_add(c, fm, ig)
            tch = sp.tile([HA, 2, BH], dt, name=f"tch{tag}", tag="tch")
            nc.scalar.activation(tch, c, Tanh)
            nc.vector.tensor_mul(ha, sgo[:, 0, :], tch[:, 0, :])
            nc.vector.tensor_mul(hb, sgo[0:HB, 1, :], tch[0:HB, 1, :])

        def emit_lstm1(hh, t, hx):
            x1_rhs = TACT[:, t, hh["cs"]] if t < CTX else hh["x1"]
            nc.vector.tensor_copy(hh["TD"][96:96 + A, :],
                                  ACTS[:, t, hh["cs"]])
            P1 = lstm_gates([("w1ha", hh["TA1"]), ("w1hb", hh["TB1"]),
                             ("w1x", x1_rhs)], f"1_{t}_{hx}")
            lstm_cell(P1, hh["c1"], hh["TA1"], hh["TB1"][0:HB, :],
                      f"1_{t}_{hx}")

        def emit_lstm2(hh, t, hx):
            P2 = lstm_gates([("w2h2a", hh["TA2"]), ("w2td", hh["TD"]),
                             ("w2h1a", hh["TA1"]), ("w2h1b", hh["TB1"])],
                            f"2_{t}_{hx}")
            lstm_cell(P2, hh["c2"], hh["TA2"], hh["TD"][0:HB, :],
                      f"2_{t}_{hx}")

        def emit_fc(hh, t, hx):
            x1_rhs = TACT[:, t, hh["cs"]] if t < CTX else hh["x1"]
            fcp = pp.tile([HA, 2, BH], f32, name=f"fcp_{t}_{hx}", tag="g")
            for m, (mo, mn) in enumerate(((0, HA), (HA, H))):
                ps = fcp[0:mn - mo, m, :]
                cols = slice(mo, mn)
                nc.tensor.matmul(ps, W["w3x"][:, cols], x1_rhs,
                                 start=True, stop=False)
                nc.tensor.matmul(ps, W["w3ha"][:, cols], hh["TA2"],
                                 start=False, stop=False)
                nc.tensor.matmul(ps, W["w3td"][:, cols], hh["TD"],
                                 start=False, stop=True)
            nc.scalar.activation(hh["TE"], fcp[:, 0, :], Tanh)
            nc.scalar.activation(hh["TF"][0:HB, :], fcp[0:HB, 1, :], Tanh)
            f2p = pp.tile([F, BH], f32, name=f"f2p_{t}_{hx}", tag="g")
            nc.tensor.matmul(f2p, W["w4a"], hh["TE"], start=True, stop=False)
            nc.tensor.matmul(f2p, W["w4b"], hh["TF"], start=False, stop=True)
            stg = op.tile([F, BH], f32, name=f"stg_{t}_{hx}", tag="stg")
            if t < NSTEP - 1:
                nc.scalar.activation(hh["x1"][0:F, :], f2p, Tanh, bias=B4)
                nc.vector.tensor_copy(stg, hh["x1"][0:F, :])
            else:
                nc.scalar.activation(stg, f2p, Tanh, bias=B4)
            nc.gpsimd.dma_start(out=out[t - (CTX - 1), :, hh["cs"]], in_=stg)

        X, Y = halves
        # zipper the two independent half-batch recurrences: each half's fc
        # block is emitted inside the other half's LSTM1 window so the PE
        # FIFO always holds runnable matmuls during dependency chains
        for t in range(NSTEP):
            emit_lstm1(X, t, 0)
            if t - 1 >= CTX - 1:
                emit_fc(Y, t - 1, 1)
            emit_lstm1(Y, t, 1)
            emit_lstm2(X, t, 0)
            emit_lstm2(Y, t, 1)
            if t >= CTX - 1:
                emit_fc(X, t, 0)
        emit_fc(Y, NSTEP - 1, 1)

    nc.finalize()
    return nc


@functools.lru_cache(maxsize=1)
def _get_nc():
    return _build_nc()


def _prep_weights(W_ih1, W_hh1, b_ih1, b_hh1, W_ih2, W_hh2, b_ih2, b_hh2,
                  fc1_w, fc1_b, fc2_w, fc2_b):
    # gate rows reordered [i, f, o, g]
    perm = np.concatenate([np.arange(0, 200), np.arange(200, 400),
                           np.arange(600, 800), np.arange(400, 600)])
    W1p = np.asarray(W_ih1)[perm]          # [800, 48]
    W1hp = np.asarray(W_hh1)[perm]         # [800, 200]
    b1p = (np.asarray(b_ih1) + np.asarray(b_hh1))[perm]
    W2p = np.asarray(W_ih2)[perm]          # [800, 248]
    W2hp = np.asarray(W_hh2)[perm]         # [800, 200]
    b2p = (np.asarray(b_ih2) + np.asarray(b_hh2))[perm]
    Wt = W2p[:, 200:248]
    W2eff = Wt[:, 0:12] + Wt[:, 12:24] + Wt[:, 24:36] + Wt[:, 36:48]  # [800,12]
    fc1_w = np.asarray(fc1_w); fc1_b = np.asarray(fc1_b)
    fc2_w = np.asarray(fc2_w); fc2_b = np.asarray(fc2_b)
    npdt = _npdt()

    def c(x):
        return np.ascontiguousarray(x).astype(npdt)

    def pad_to(x, k):
        z = np.zeros((k, x.shape[1]), np.float32)
        z[:x.shape[0]] = x
        return z

    def td_weights(w_hb_T, w_as_T, bias):
        z = np.zeros((KD, w_hb_T.shape[1]), np.float32)
        z[0:HB] = w_hb_T
        z[96:108] = w_as_T
        z[108] = bias
        return z

    return {
        "w1ha": c(W1hp[:, 0:HA].T),
        "w1hb": c(pad_to(W1hp[:, HA:H].T, KB)),
        "w1x": c(pad_to(np.concatenate([W1p.T, b1p[None, :]], 0), KX)),
        "w2h2a": c(W2hp[:, 0:HA].T),
        "w2td": c(td_weights(W2hp[:, HA:H].T, W2eff.T, b2p)),
        "w2h1a": c(W2p[:, 0:HA].T),
        "w2h1b": c(pad_to(W2p[:, HA:H].T, KB)),
        "w3ha": c(fc1_w[:, 0:HA].T),
        "w3td": c(pad_to(fc1_w[:, HA:H].T, KD)),
        "w3x": c(pad_to(np.concatenate([fc1_w[:, 200:248].T,
                                        fc1_b[None, :]], 0), KX)),
        "w4a": c(fc2_w[:, 0:HA].T),
        "w4b": c(pad_to(fc2_w[:, HA:H].T, 96)),
        "b4": np.ascontiguousarray(fc2_b[:, None]).astype(np.float32),
    }


def kernel(tactiles, actions, W_ih1, W_hh1, b_ih1, b_hh1,
           W_ih2, W_hh2, b_ih2, b_hh2, fc1_w, fc1_b, fc2_w, fc2_b):
    global LAST_RESULT
    npdt = _npdt()
    tactiles = np.asarray(tactiles)
    actions = np.asarray(actions)

    wmap = _prep_weights(W_ih1, W_hh1, b_ih1, b_hh1, W_ih2, W_hh2, b_ih2, b_hh2,
                         fc1_w, fc1_b, fc2_w, fc2_b)

    in_maps = []
    for i in range(NCORES):
        s = slice(i * BL, (i + 1) * BL)
        tt = np.zeros((KX, CTX, BL), np.float32)
        tt[0:F] = tactiles[0:CTX, s, :].transpose(2, 0, 1)
        tt[F] = 1.0
        acts_T = np.ascontiguousarray(
            actions[1:T, s, :].transpose(2, 0, 1)).astype(npdt)   # [6,119,BL]
        sz = np.zeros((64, BL), np.float32)
        sz[0] = 1.0                      # x1 ones row
        sz[8:8 + A] = actions[0, s, :].T  # state rows
        sz[8 + A] = 1.0                  # TD ones row
        m = {"tact": tt.astype(npdt), "acts": acts_T,
             "statzero": sz.astype(npdt)}
        m.update(wmap)
        in_maps.append(m)

    nc = _get_nc()
    res = run_bass_kernel_spmd(nc, in_maps, core_ids=list(range(NCORES)))
    LAST_RESULT = res

    outs = [np.asarray(r["out"], dtype=np.float32) for r in res.results]
    full = np.concatenate([o.transpose(0, 2, 1) for o in outs], axis=1)
    return np.ascontiguousarray(full)
